# revision 2
# baseline (speedup 1.0000x reference)
"""Trainium2 Bass kernel for an 8-head self-attention block (MHA).

Problem: x[2, 4096, 512], 8 heads x 64 dims, torch-Linear q/k/v/o projections,
softmax attention, residual:  out = softmax(q k^T / 8) v @ Wo^T + bo + x.

Sharding (8 NeuronCores, no collectives): core c handles batch b = c // 4 and
query rows (c % 4) * 1024 ... + 1024, for ALL heads.  K/V for the full
sequence are computed on every core of a batch group (projections are cheap
relative to attention), so the output projection is fully local to a core.

Engine budget per core (the design drivers):
  - ACT (exp) is the largest stream: 8 heads x 32 s-chunks x [128, 1024]
    activations = 262144 elem/partition ~ 270 us if ACT does all of it.
  - The score matmuls contract over only D=64, so two heads packed in the
    two 64-row halves of the PE array (tile_position row groups 0 / 64)
    run CONCURRENTLY - heads are processed in pairs (2p, 2p+1).
  - For pairs 1-3, the odd head's exp runs on the otherwise-idle Vector
    engine via a Schraudolph bit-trick: i16 = round(x * 128/ln2 + (127*128
    - c)) reinterpreted as bf16 is exp(x) to ~3%; softmax cancels the
    constant bias, and the residual noise on 3 of 8 heads is ~1e-3 of the
    output. This takes ~90 us off the ACT critical path.

Per-pair dataflow (PSUM: 2 score regions x 2 banks + 2 PV accumulators x
2 banks = 8 banks):
  - scores^T chunk [s=128, q=1024] per head = kT_sl.T @ qT_sl on PE
    (f32 psum), interleaved A/B halves so the row-groups overlap
  - exp -> P~ bf16 (no max subtraction: scores are O(1) here)
  - PV: lhsT = P~ [s=128, q=128], rhs = [V | 1] [s=128, 65] -> accumulates
    o[q, 65], softmax denominator in psum column 64; one chunk behind the
    scores so PE never waits on the exp
  - normalize = reciprocal + tensor_scalar multiply on DVE, deferred into
    the next pair's stream; o tiles PE-transposed into oT[f, q] for the
    output projection; out bias folded into the residual host-side.
"""

import numpy as np

B = 2
S = 4096
E = 512
H = 8
D = 64
P = 128
EC = E // P          # 4 e-chunks
FC = E // P          # 4 f-chunks
NJ = S // P          # 32 s-chunks
QR = S // 4          # 1024 query rows per core
NQS = QR // 512      # 2 query strips of 512
NKS = S // 512       # 8 s-strips of 512
NPAIR = H // 2       # 4 head pairs
NQC = QR // P        # 8 query chunks of 128

# Schraudolph exp-as-bf16-bits constants (bf16 has 7 mantissa bits)
EXP_A = float(128.0 / np.log(2.0))
EXP_B = float(127.0 * 128.0 - 6.9)

_CACHE = {}


def _build_nc():
    import concourse.bass as bass
    import concourse.tile as tile
    from concourse import bacc, mybir

    f32 = mybir.dt.float32
    bf16 = mybir.dt.bfloat16
    i16 = mybir.dt.int16
    AFT = mybir.ActivationFunctionType
    Alu = mybir.AluOpType

    nc = bacc.Bacc("TRN2", target_bir_lowering=False, debug=False, num_devices=8)

    xT_d = nc.declare_dram_parameter("xT", [E, S], bf16, isOutput=False)
    xqT_d = nc.declare_dram_parameter("xqT", [E, QR], bf16, isOutput=False)
    xres_d = nc.declare_dram_parameter("xres", [QR, E], f32, isOutput=False)
    wqT_d = nc.declare_dram_parameter("wqT", [E, E], bf16, isOutput=False)
    wkT_d = nc.declare_dram_parameter("wkT", [E, E], bf16, isOutput=False)
    wvT_d = nc.declare_dram_parameter("wvT", [E, E], bf16, isOutput=False)
    woT_d = nc.declare_dram_parameter("woT", [E, E], bf16, isOutput=False)
    bq_d = nc.declare_dram_parameter("bq", [P, FC], f32, isOutput=False)
    bk_d = nc.declare_dram_parameter("bk", [P, FC], f32, isOutput=False)
    bv_d = nc.declare_dram_parameter("bv", [E], f32, isOutput=False)
    ident_d = nc.declare_dram_parameter("ident", [P, P], bf16, isOutput=False)
    out_d = nc.declare_dram_parameter("out", [QR, E], f32, isOutput=True)

    with tile.TileContext(nc) as tc:
        with tc.tile_pool(name="const", bufs=1) as const, \
             tc.tile_pool(name="persist", bufs=1) as persist:

            # ---- constants that live for the whole kernel ----
            wo_sb = const.tile([P, EC, E], bf16)
            bq_sb = const.tile([P, FC], f32)
            bk_sb = const.tile([P, FC], f32)
            bv_sb = const.tile([P, E], f32)
            # identity for PE transposes (loaded after the projection
            # phase - see below - to keep startup DMA on the critical path)
            ident_sb = const.tile([P, P], bf16)
            # residual rows (+ output bias, folded host-side)
            xres_sb = const.tile([P, QR // P, E], f32)

            # ---- persistent activations ----
            kT_sb = persist.tile([P, FC, S], bf16)           # 32 KB/p
            qT_sb = persist.tile([P, FC, QR], bf16)          # 8 KB/p
            v_sb = persist.tile([P, NJ, H, 65], bf16)        # 32.5 KB/p
            oT_sb = persist.tile([P, FC, QR], bf16)          # 8 KB/p

            # constant-1 columns (softmax denominator trick)
            nc.vector.memset(v_sb[:, :, :, 64:65], 1.0)

            with tc.tile_pool(name="wpool", bufs=1) as wpool, \
                 tc.tile_pool(name="xtp", bufs=3) as xtp, \
                 tc.tile_pool(name="work", bufs=4) as work, \
                 tc.tile_pool(name="opool", bufs=2) as opool, \
                 tc.tile_pool(name="ps_sc", bufs=2, space="PSUM") as ps_sc, \
                 tc.tile_pool(name="ps_pv", bufs=2, space="PSUM") as ps_pv:

                wq_sb = wpool.tile([P, EC, E], bf16)
                wk_sb = wpool.tile([P, EC, E], bf16)
                wv_sb = wpool.tile([P, EC, E], bf16)
                # per-e-chunk loads so the first matmul only waits for
                # the first 128 rows of Wq rather than the whole tensor
                for t, d in ((wq_sb, wqT_d), (wk_sb, wkT_d), (wv_sb, wvT_d)):
                    for e in range(EC):
                        nc.sync.dma_start(
                            out=t[:, e, :], in_=d[e * P:(e + 1) * P, :])
                nc.sync.dma_start(out=bq_sb[:], in_=bq_d[:])
                nc.sync.dma_start(out=bk_sb[:], in_=bk_d[:])
                nc.sync.dma_start(
                    out=bv_sb[:],
                    in_=bass.AP(tensor=bv_d, offset=0, ap=[[0, P], [1, E]]))

                # B2: qT[f, q] = (Wq @ xq^T + bq) / 8 (first: C needs it all)
                for qs in range(NQS):
                    qsl = slice(qs * 512, (qs + 1) * 512)
                    xq = xtp.tile([P, EC, 512], bf16, tag="xt")
                    for e in range(EC):
                        nc.sync.dma_start(
                            out=xq[:, e, :], in_=xqT_d[e * P:(e + 1) * P, qsl])
                    for f in range(FC):
                        pq = ps_sc.tile([P, 512], f32, tag="sc", name="pq")
                        for e in range(EC):
                            nc.tensor.matmul(
                                pq[:], wq_sb[:, e, f * P:(f + 1) * P],
                                xq[:, e, :], start=(e == 0), stop=(e == EC - 1),
                                skip_group_check=True)
                        nc.vector.tensor_scalar(
                            qT_sb[:, f, qsl], pq[:], bq_sb[:, f:f + 1],
                            float(1.0 / np.sqrt(D)), Alu.add, Alu.mult)

                # ---- attention, head pairs on PE row groups 0 / 64 ----

                def emit_normalize(stg, h, also_d=False, qcs=None):
                    fc = h // 2
                    fr = (h % 2) * 64
                    qcs = range(NQC) if qcs is None else qcs
                    rcp = opool.tile([P, NQC, 1], f32, tag="rcp", name="rcp",
                                     bufs=2)
                    nc.vector.reciprocal(rcp[:, qcs[0]:qcs[-1] + 1, :],
                                         stg[:, qcs[0]:qcs[-1] + 1, 64:65])
                    o_sb = opool.tile([P, NQC, 64], bf16, tag="o", name="o_sb",
                                      bufs=2)
                    for qc in qcs:
                        nc.vector.tensor_scalar_mul(
                            o_sb[:, qc, :], stg[:, qc, 0:64], rcp[:, qc, :])
                    for qc in qcs:
                        # transpose [128 q, 64 d] -> [64 d, 128 q] on PE,
                        # directly at the head's partition base
                        tp = ps_sc.tile([P, P], bf16, tag="sc", name="tp")
                        nc.tensor.transpose(tp[fr:fr + 64, :], o_sb[:, qc, :],
                                            ident_sb[:])
                        nc.vector.tensor_copy(
                            oT_sb[fr:fr + 64, fc, qc * P:(qc + 1) * P],
                            tp[fr:fr + 64, :])
                        if also_d:
                            # last head: output projection for this q-chunk
                            # follows immediately (all other heads' oT pieces
                            # already landed), overlapping the projection with
                            # the tail of attention
                            po = ps_sc.tile([P, E], f32, tag="sc", name="po")
                            for e in range(EC):
                                nc.tensor.matmul(
                                    po[:], oT_sb[:, e, qc * P:(qc + 1) * P],
                                    wo_sb[:, e, :], start=(e == 0),
                                    stop=(e == EC - 1), skip_group_check=True)
                            ot = opool.tile([P, E], f32, tag="ot", name="ot")
                            nc.vector.tensor_add(ot[:], po[:],
                                                 xres_sb[:, qc, :])
                            nc.sync.dma_start(
                                out=out_d[qc * P:(qc + 1) * P, :], in_=ot[:])

                def emit_pair_chunk(p, j, pvpA, pvpB, prev, dve_b):
                    """Scores+exp for pair p chunk j; PV for chunk j-1.

                    The A/B score matmuls are interleaved so adjacent PE
                    instructions target different row groups (0 / 64) and
                    execute concurrently in the array.
                    """
                    hA, hB = 2 * p, 2 * p + 1
                    scA = ps_sc.tile([P, QR], f32, tag="sc", name="scA")
                    scB = ps_sc.tile([P, QR], f32, tag="sc", name="scB")
                    for hf in range(QR // 512):
                        hsl = slice(hf * 512, (hf + 1) * 512)
                        nc.tensor.matmul(
                            scA[:, hsl], kT_sb[0:64, p, j * P:(j + 1) * P],
                            qT_sb[0:64, p, hsl],
                            start=True, stop=True, skip_group_check=True)
                        nc.tensor.matmul(
                            scB[:, hsl], kT_sb[64:128, p, j * P:(j + 1) * P],
                            qT_sb[64:128, p, hsl],
                            start=True, stop=True, skip_group_check=True)
                    ptA = work.tile([P, QR], bf16, tag="pt", name="ptA")
                    nc.scalar.activation(ptA[:], scA[:], AFT.Exp)
                    if dve_b:
                        ptB = work.tile([P, QR], i16, tag="pt", name="ptB")
                        nc.vector.tensor_scalar(
                            ptB[:], scB[:], EXP_A, EXP_B, Alu.mult, Alu.add)
                    else:
                        ptB = work.tile([P, QR], bf16, tag="pt", name="ptB")
                        nc.scalar.activation(ptB[:], scB[:], AFT.Exp)
                    # software pipeline: PV for chunk j-1 is emitted after the
                    # scores matmuls of chunk j so PE never waits on the exp
                    if prev is not None:
                        pA, pB, pj = prev
                        for qc in range(NQC):
                            # a start=True matmul clears its whole psum BANK's
                            # has_written bits, so only the first region per
                            # bank (qc 0 and 4) sets it; the other regions'
                            # first writes then overwrite stale data instead
                            # of accumulating onto it
                            nc.tensor.matmul(
                                pvpA[:, qc, 0:65],
                                pA[:, qc * P:(qc + 1) * P].bitcast(bf16),
                                v_sb[:, pj, hA, :],
                                start=(pj == 0 and qc % 4 == 0),
                                stop=False, skip_group_check=True)
                        for qc in range(NQC):
                            nc.tensor.matmul(
                                pvpB[:, qc, 0:65],
                                pB[:, qc * P:(qc + 1) * P].bitcast(bf16),
                                v_sb[:, pj, hB, :],
                                start=(pj == 0 and qc % 4 == 0),
                                stop=False, skip_group_check=True)
                    return (ptA[:], ptB[:], j)

                def finish_pair(p, pvpA, pvpB, prev):
                    hA, hB = 2 * p, 2 * p + 1
                    pA, pB, pj = prev
                    for qc in range(NQC):
                        nc.tensor.matmul(
                            pvpA[:, qc, 0:65],
                            pA[:, qc * P:(qc + 1) * P].bitcast(bf16),
                            v_sb[:, pj, hA, :], start=False, stop=True,
                            skip_group_check=True)
                    for qc in range(NQC):
                        nc.tensor.matmul(
                            pvpB[:, qc, 0:65],
                            pB[:, qc * P:(qc + 1) * P].bitcast(bf16),
                            v_sb[:, pj, hB, :], start=False, stop=True,
                            skip_group_check=True)
                    # staging copies (one per psum bank) free the psum
                    # accumulators almost immediately
                    stgA = opool.tile([P, NQC, 65], f32, tag="stg",
                                      name="stgA", bufs=4)
                    nc.vector.tensor_copy(stgA[:, 0:4], pvpA[:, 0:4, 0:65])
                    nc.vector.tensor_copy(stgA[:, 4:8], pvpA[:, 4:8, 0:65])
                    stgB = opool.tile([P, NQC, 65], f32, tag="stg",
                                      name="stgB", bufs=4)
                    nc.vector.tensor_copy(stgB[:, 0:4], pvpB[:, 0:4, 0:65])
                    nc.vector.tensor_copy(stgB[:, 4:8], pvpB[:, 4:8, 0:65])
                    return (stgA, stgB, p)

                # B1+B3+pair-0 interleaved: kT strips and V chunks come from
                # the same xt tile, and pair 0's scores/exp/PV for a strip's
                # four chunks follow immediately, so the ACT exp pipeline
                # starts ~70us earlier and fills projection DMA gaps
                pvpA0 = ps_pv.tile([P, NQC, P], f32, tag="pv", name="pvpA")
                pvpB0 = ps_pv.tile([P, NQC, P], f32, tag="pv", name="pvpB")
                prev = None
                for strip in range(NKS):
                    ssl = slice(strip * 512, (strip + 1) * 512)
                    xt = xtp.tile([P, EC, 512], bf16, tag="xt")
                    for e in range(EC):
                        nc.sync.dma_start(
                            out=xt[:, e, :], in_=xT_d[e * P:(e + 1) * P, ssl])
                    for f in range(FC):
                        pk = ps_sc.tile([P, 512], f32, tag="sc", name="pk")
                        for e in range(EC):
                            nc.tensor.matmul(
                                pk[:], wk_sb[:, e, f * P:(f + 1) * P],
                                xt[:, e, :], start=(e == 0), stop=(e == EC - 1),
                                skip_group_check=True)
                        nc.vector.tensor_scalar_add(
                            kT_sb[:, f, ssl], pk[:], bk_sb[:, f:f + 1])
                    for k in range(4):
                        j = strip * 4 + k
                        pv = ps_sc.tile([P, E], f32, tag="sc", name="pvx")
                        for e in range(EC):
                            nc.tensor.matmul(
                                pv[:], xt[:, e, k * P:(k + 1) * P],
                                wv_sb[:, e, :], start=(e == 0),
                                stop=(e == EC - 1), skip_group_check=True)
                        pv_v = pv[:].rearrange("p (h d) -> p h d", h=H)
                        bv_v = bv_sb[:].rearrange("p (h d) -> p h d", h=H)
                        nc.vector.tensor_add(v_sb[:, j, :, 0:64], pv_v[:],
                                             bv_v[:])
                    for k in range(4):
                        prev = emit_pair_chunk(0, strip * 4 + k, pvpA0, pvpB0,
                                               prev, dve_b=False)
                pending = finish_pair(0, pvpA0, pvpB0, prev)

                # tail-only data: loaded now, off the startup critical path
                nc.sync.dma_start(out=ident_sb[:], in_=ident_d[:])
                nc.sync.dma_start(
                    out=wo_sb[:],
                    in_=woT_d.ap().rearrange("(c p) f -> p c f", p=P))
                nc.sync.dma_start(
                    out=xres_sb[:],
                    in_=xres_d.ap().rearrange("(k p) f -> p k f", p=P))

                for p in range(1, NPAIR):
                    pvpA = ps_pv.tile([P, NQC, P], f32, tag="pv", name="pvpA")
                    pvpB = ps_pv.tile([P, NQC, P], f32, tag="pv", name="pvpB")
                    prev = None
                    for j in range(NJ):
                        prev = emit_pair_chunk(p, j, pvpA, pvpB, prev,
                                               dve_b=True)
                        if pending is not None and j in (6, 12, 18, 24):
                            # deferred: previous pair's normalize runs inside
                            # this pair's stream, long after its inputs
                            # landed, in four half-blocks to spread the PE
                            # transpose work
                            stgA, stgB, pp = pending
                            half = range(NQC // 2) if j in (6, 18) \
                                else range(NQC // 2, NQC)
                            src, hh = (stgA, 2 * pp) if j in (6, 12) \
                                else (stgB, 2 * pp + 1)
                            emit_normalize(src, hh, qcs=half)
                            if j == 24:
                                pending = None
                    pending = finish_pair(p, pvpA, pvpB, prev)

                stgA, stgB, pp = pending
                emit_normalize(stgA, 2 * pp)
                emit_normalize(stgB, 2 * pp + 1, also_d=True)

    nc.compile()
    return nc


def _get_nc():
    if "nc" not in _CACHE:
        _CACHE["nc"] = _build_nc()
    return _CACHE["nc"]


def run_spmd(in_maps, **kw):
    from concourse.bass_utils import run_bass_kernel_spmd
    nc = _get_nc()
    return run_bass_kernel_spmd(nc, in_maps, list(range(8)), **kw)


def make_in_maps(x, Wq, bq, Wk, bk, Wv, bv, Wo, bo):
    import ml_dtypes
    bf = ml_dtypes.bfloat16
    x = np.asarray(x, dtype=np.float32)
    f32c = lambda a: np.ascontiguousarray(np.asarray(a, dtype=np.float32))
    bfc = lambda a: np.ascontiguousarray(
        np.asarray(a, dtype=np.float32).astype(bf))
    wqT = bfc(np.asarray(Wq).T)
    wkT = bfc(np.asarray(Wk).T)
    wvT = bfc(np.asarray(Wv).T)
    woT = bfc(np.asarray(Wo).T)
    bq_r = f32c(np.asarray(bq).reshape(FC, P).T)
    bk_r = f32c(np.asarray(bk).reshape(FC, P).T)
    bv_a = f32c(bv)
    bo_a = np.asarray(bo, dtype=np.float32)
    ident = np.eye(P, dtype=np.float32).astype(bf)
    xT = [bfc(x[b].T) for b in range(B)]

    in_maps = []
    for c in range(8):
        b, r = c // 4, c % 4
        in_maps.append({
            "xT": xT[b],
            "xqT": np.ascontiguousarray(xT[b][:, r * QR:(r + 1) * QR]),
            # output bias folded into the residual tile (host-side, free)
            "xres": f32c(x[b, r * QR:(r + 1) * QR] + bo_a),
            "wqT": wqT, "wkT": wkT, "wvT": wvT, "woT": woT,
            "bq": bq_r, "bk": bk_r, "bv": bv_a,
            "ident": ident,
        })
    return in_maps


def assemble(results):
    out = np.empty((B, S, E), dtype=np.float32)
    for c in range(8):
        b, r = c // 4, c % 4
        out[b, r * QR:(r + 1) * QR] = results[c]["out"]
    return out


def kernel(x, Wq, bq, Wk, bk, Wv, bv, Wo, bo):
    in_maps = make_in_maps(x, Wq, bq, Wk, bk, Wv, bv, Wo, bo)
    res = run_spmd(in_maps)
    return assemble(res.results)


# revision 3
# speedup vs baseline: 1.0278x; 1.0278x over previous
"""Trainium2 Bass kernel for an 8-head self-attention block (MHA).

Problem: x[2, 4096, 512], 8 heads x 64 dims, torch-Linear q/k/v/o projections,
softmax attention, residual:  out = softmax(q k^T / 8) v @ Wo^T + bo + x.

Sharding (8 NeuronCores, no collectives): core c handles batch b = c // 4 and
query rows (c % 4) * 1024 ... + 1024, for ALL heads.  K/V for the full
sequence are computed on every core of a batch group (projections are cheap
relative to attention), so the output projection is fully local to a core.

Engine budget per core (the design drivers):
  - ACT (exp) is the largest stream: 8 heads x 32 s-chunks x [128, 1024]
    activations = 262144 elem/partition ~ 270 us if ACT does all of it.
  - The score matmuls contract over only D=64, so two heads packed in the
    two 64-row halves of the PE array (tile_position row groups 0 / 64)
    run CONCURRENTLY - heads are processed in pairs (2p, 2p+1).
  - For pairs 1-3, the odd head's exp runs on the otherwise-idle Vector
    engine via a Schraudolph bit-trick: i16 = round(x * 128/ln2 + (127*128
    - c)) reinterpreted as bf16 is exp(x) to ~3%; softmax cancels the
    constant bias, and the residual noise on 3 of 8 heads is ~1e-3 of the
    output. This takes ~90 us off the ACT critical path.

Per-pair dataflow (PSUM: 2 score regions x 2 banks + 2 PV accumulators x
2 banks = 8 banks):
  - scores^T chunk [s=128, q=1024] per head = kT_sl.T @ qT_sl on PE
    (f32 psum), interleaved A/B halves so the row-groups overlap
  - exp -> P~ bf16 (no max subtraction: scores are O(1) here)
  - PV: lhsT = P~ [s=128, q=128], rhs = [V | 1] [s=128, 65] -> accumulates
    o[q, 65], softmax denominator in psum column 64; one chunk behind the
    scores so PE never waits on the exp
  - normalize = reciprocal + tensor_scalar multiply on DVE, deferred into
    the next pair's stream; o tiles PE-transposed into oT[f, q] for the
    output projection; out bias folded into the residual host-side.
"""

import numpy as np

B = 2
S = 4096
E = 512
H = 8
D = 64
P = 128
EC = E // P          # 4 e-chunks
FC = E // P          # 4 f-chunks
NJ = S // P          # 32 s-chunks
QR = S // 4          # 1024 query rows per core
NQS = QR // 512      # 2 query strips of 512
NKS = S // 512       # 8 s-strips of 512
NPAIR = H // 2       # 4 head pairs
NQC = QR // P        # 8 query chunks of 128

# Schraudolph exp-as-bf16-bits constants (bf16 has 7 mantissa bits)
EXP_A = float(128.0 / np.log(2.0))
EXP_B = float(127.0 * 128.0 - 6.9)

_CACHE = {}


def _build_nc():
    import concourse.bass as bass
    import concourse.tile as tile
    from concourse import bacc, mybir

    f32 = mybir.dt.float32
    bf16 = mybir.dt.bfloat16
    i16 = mybir.dt.int16
    AFT = mybir.ActivationFunctionType
    Alu = mybir.AluOpType

    nc = bacc.Bacc("TRN2", target_bir_lowering=False, debug=False, num_devices=8)

    xT_d = nc.declare_dram_parameter("xT", [E, S], bf16, isOutput=False)
    xqT_d = nc.declare_dram_parameter("xqT", [E, QR], bf16, isOutput=False)
    xres_d = nc.declare_dram_parameter("xres", [QR, E], f32, isOutput=False)
    wqT_d = nc.declare_dram_parameter("wqT", [E, E], bf16, isOutput=False)
    wkT_d = nc.declare_dram_parameter("wkT", [E, E], bf16, isOutput=False)
    wvT_d = nc.declare_dram_parameter("wvT", [E, E], bf16, isOutput=False)
    woT_d = nc.declare_dram_parameter("woT", [E, E], bf16, isOutput=False)
    bq_d = nc.declare_dram_parameter("bq", [P, FC], f32, isOutput=False)
    bk_d = nc.declare_dram_parameter("bk", [P, FC], f32, isOutput=False)
    bv_d = nc.declare_dram_parameter("bv", [E], f32, isOutput=False)
    ident_d = nc.declare_dram_parameter("ident", [P, P], bf16, isOutput=False)
    out_d = nc.declare_dram_parameter("out", [QR, E], f32, isOutput=True)

    with tile.TileContext(nc) as tc:
        with tc.tile_pool(name="const", bufs=1) as const, \
             tc.tile_pool(name="persist", bufs=1) as persist:

            # ---- constants that live for the whole kernel ----
            wo_sb = const.tile([P, EC, E], bf16)
            bq_sb = const.tile([P, FC], f32)
            bk_sb = const.tile([P, FC], f32)
            bv_sb = const.tile([P, E], f32)
            # identity for PE transposes (loaded after the projection
            # phase - see below - to keep startup DMA on the critical path)
            ident_sb = const.tile([P, P], bf16)
            # residual rows (+ output bias, folded host-side)
            xres_sb = const.tile([P, QR // P, E], f32)

            # ---- persistent activations ----
            kT_sb = persist.tile([P, FC, S], bf16)           # 32 KB/p
            qT_sb = persist.tile([P, FC, QR], bf16)          # 8 KB/p
            v_sb = persist.tile([P, NJ, H, 65], bf16)        # 32.5 KB/p
            oT_sb = persist.tile([P, FC, QR], bf16)          # 8 KB/p

            # constant-1 columns (softmax denominator trick)
            nc.vector.memset(v_sb[:, :, :, 64:65], 1.0)

            with tc.tile_pool(name="wpool", bufs=1) as wpool, \
                 tc.tile_pool(name="xtp", bufs=3) as xtp, \
                 tc.tile_pool(name="work", bufs=4) as work, \
                 tc.tile_pool(name="opool", bufs=2) as opool, \
                 tc.tile_pool(name="ps_sc", bufs=2, space="PSUM") as ps_sc, \
                 tc.tile_pool(name="ps_pv", bufs=2, space="PSUM") as ps_pv:

                wq_sb = wpool.tile([P, EC, E], bf16)
                wk_sb = wpool.tile([P, EC, E], bf16)
                wv_sb = wpool.tile([P, EC, E], bf16)
                # per-e-chunk loads so the first matmul only waits for
                # the first 128 rows of Wq rather than the whole tensor
                for t, d in ((wq_sb, wqT_d), (wk_sb, wkT_d), (wv_sb, wvT_d)):
                    for e in range(EC):
                        nc.sync.dma_start(
                            out=t[:, e, :], in_=d[e * P:(e + 1) * P, :])
                nc.sync.dma_start(out=bq_sb[:], in_=bq_d[:])
                nc.sync.dma_start(out=bk_sb[:], in_=bk_d[:])
                nc.sync.dma_start(
                    out=bv_sb[:],
                    in_=bass.AP(tensor=bv_d, offset=0, ap=[[0, P], [1, E]]))

                # B2: qT[f, q] = (Wq @ xq^T + bq) / 8 (first: C needs it all)
                for qs in range(NQS):
                    qsl = slice(qs * 512, (qs + 1) * 512)
                    xq = xtp.tile([P, EC, 512], bf16, tag="xt")
                    for e in range(EC):
                        nc.sync.dma_start(
                            out=xq[:, e, :], in_=xqT_d[e * P:(e + 1) * P, qsl])
                    for f in range(FC):
                        pq = ps_sc.tile([P, 512], f32, tag="sc", name="pq")
                        for e in range(EC):
                            nc.tensor.matmul(
                                pq[:], wq_sb[:, e, f * P:(f + 1) * P],
                                xq[:, e, :], start=(e == 0), stop=(e == EC - 1),
                                skip_group_check=True)
                        nc.vector.tensor_scalar(
                            qT_sb[:, f, qsl], pq[:], bq_sb[:, f:f + 1],
                            float(1.0 / np.sqrt(D)), Alu.add, Alu.mult)

                # ---- attention, head pairs on PE row groups 0 / 64 ----

                def emit_normalize(stg, h, also_d=False, qcs=None):
                    fc = h // 2
                    fr = (h % 2) * 64
                    qcs = range(NQC) if qcs is None else qcs
                    rcp = opool.tile([P, NQC, 1], f32, tag="rcp", name="rcp",
                                     bufs=2)
                    nc.vector.reciprocal(rcp[:, qcs[0]:qcs[-1] + 1, :],
                                         stg[:, qcs[0]:qcs[-1] + 1, 64:65])
                    o_sb = opool.tile([P, NQC, 64], bf16, tag="o", name="o_sb",
                                      bufs=2)
                    for qc in qcs:
                        nc.vector.tensor_scalar_mul(
                            o_sb[:, qc, :], stg[:, qc, 0:64], rcp[:, qc, :])
                    for qc in qcs:
                        # transpose [128 q, 64 d] -> [64 d, 128 q] on PE,
                        # directly at the head's partition base
                        tp = ps_sc.tile([P, P], bf16, tag="sc", name="tp")
                        nc.tensor.transpose(tp[fr:fr + 64, :], o_sb[:, qc, :],
                                            ident_sb[:])
                        nc.vector.tensor_copy(
                            oT_sb[fr:fr + 64, fc, qc * P:(qc + 1) * P],
                            tp[fr:fr + 64, :])
                        if also_d:
                            # last head: output projection for this q-chunk
                            # follows immediately (all other heads' oT pieces
                            # already landed), overlapping the projection with
                            # the tail of attention
                            po = ps_sc.tile([P, E], f32, tag="sc", name="po")
                            for e in range(EC):
                                nc.tensor.matmul(
                                    po[:], oT_sb[:, e, qc * P:(qc + 1) * P],
                                    wo_sb[:, e, :], start=(e == 0),
                                    stop=(e == EC - 1), skip_group_check=True)
                            ot = opool.tile([P, E], f32, tag="ot", name="ot")
                            nc.vector.tensor_add(ot[:], po[:],
                                                 xres_sb[:, qc, :])
                            nc.sync.dma_start(
                                out=out_d[qc * P:(qc + 1) * P, :], in_=ot[:])

                def emit_pair_chunk(p, j, pvpA, pvpB, prev, dve_b):
                    """Scores+exp for pair p chunk j; PV for chunk j-1.

                    The A/B score matmuls are interleaved so adjacent PE
                    instructions target different row groups (0 / 64) and
                    execute concurrently in the array.
                    """
                    hA, hB = 2 * p, 2 * p + 1
                    scA = ps_sc.tile([P, QR], f32, tag="sc", name="scA")
                    scB = ps_sc.tile([P, QR], f32, tag="sc", name="scB")
                    for hf in range(QR // 512):
                        hsl = slice(hf * 512, (hf + 1) * 512)
                        nc.tensor.matmul(
                            scA[:, hsl], kT_sb[0:64, p, j * P:(j + 1) * P],
                            qT_sb[0:64, p, hsl],
                            start=True, stop=True, skip_group_check=True)
                        nc.tensor.matmul(
                            scB[:, hsl], kT_sb[64:128, p, j * P:(j + 1) * P],
                            qT_sb[64:128, p, hsl],
                            start=True, stop=True, skip_group_check=True)
                    ptA = work.tile([P, QR], bf16, tag="pt", name="ptA")
                    nc.scalar.activation(ptA[:], scA[:], AFT.Exp)
                    if dve_b:
                        ptB = work.tile([P, QR], i16, tag="pt", name="ptB")
                        nc.vector.tensor_scalar(
                            ptB[:], scB[:], EXP_A, EXP_B, Alu.mult, Alu.add)
                    else:
                        ptB = work.tile([P, QR], bf16, tag="pt", name="ptB")
                        nc.scalar.activation(ptB[:], scB[:], AFT.Exp)
                    # software pipeline: PV for chunk j-1 is emitted after the
                    # scores matmuls of chunk j so PE never waits on the exp
                    if prev is not None:
                        pA, pB, pj = prev
                        for qc in range(NQC):
                            # a start=True matmul clears its whole psum BANK's
                            # has_written bits, so only the first region per
                            # bank (qc 0 and 4) sets it; the other regions'
                            # first writes then overwrite stale data instead
                            # of accumulating onto it
                            nc.tensor.matmul(
                                pvpA[:, qc, 0:65],
                                pA[:, qc * P:(qc + 1) * P].bitcast(bf16),
                                v_sb[:, pj, hA, :],
                                start=(pj == 0 and qc % 4 == 0),
                                stop=False, skip_group_check=True)
                        for qc in range(NQC):
                            nc.tensor.matmul(
                                pvpB[:, qc, 0:65],
                                pB[:, qc * P:(qc + 1) * P].bitcast(bf16),
                                v_sb[:, pj, hB, :],
                                start=(pj == 0 and qc % 4 == 0),
                                stop=False, skip_group_check=True)
                    return (ptA[:], ptB[:], j)

                def finish_pair(p, pvpA, pvpB, prev):
                    hA, hB = 2 * p, 2 * p + 1
                    pA, pB, pj = prev
                    for qc in range(NQC):
                        nc.tensor.matmul(
                            pvpA[:, qc, 0:65],
                            pA[:, qc * P:(qc + 1) * P].bitcast(bf16),
                            v_sb[:, pj, hA, :], start=False, stop=True,
                            skip_group_check=True)
                    for qc in range(NQC):
                        nc.tensor.matmul(
                            pvpB[:, qc, 0:65],
                            pB[:, qc * P:(qc + 1) * P].bitcast(bf16),
                            v_sb[:, pj, hB, :], start=False, stop=True,
                            skip_group_check=True)
                    # staging copies (one per psum bank) free the psum
                    # accumulators almost immediately
                    stgA = opool.tile([P, NQC, 65], f32, tag="stg",
                                      name="stgA", bufs=4)
                    nc.vector.tensor_copy(stgA[:, 0:4], pvpA[:, 0:4, 0:65])
                    nc.vector.tensor_copy(stgA[:, 4:8], pvpA[:, 4:8, 0:65])
                    stgB = opool.tile([P, NQC, 65], f32, tag="stg",
                                      name="stgB", bufs=4)
                    nc.vector.tensor_copy(stgB[:, 0:4], pvpB[:, 0:4, 0:65])
                    nc.vector.tensor_copy(stgB[:, 4:8], pvpB[:, 4:8, 0:65])
                    return (stgA, stgB, p)

                # B1+B3+pair-0 interleaved: kT strips and V chunks come from
                # the same xt tile, and pair 0's scores/exp/PV for a strip's
                # four chunks follow immediately, so the ACT exp pipeline
                # starts ~70us earlier and fills projection DMA gaps
                pvpA0 = ps_pv.tile([P, NQC, P], f32, tag="pv", name="pvpA")
                pvpB0 = ps_pv.tile([P, NQC, P], f32, tag="pv", name="pvpB")
                prev = None
                for strip in range(NKS):
                    ssl = slice(strip * 512, (strip + 1) * 512)
                    xt = xtp.tile([P, EC, 512], bf16, tag="xt")
                    for e in range(EC):
                        nc.sync.dma_start(
                            out=xt[:, e, :], in_=xT_d[e * P:(e + 1) * P, ssl])
                    for f in range(FC):
                        pk = ps_sc.tile([P, 512], f32, tag="sc", name="pk")
                        for e in range(EC):
                            nc.tensor.matmul(
                                pk[:], wk_sb[:, e, f * P:(f + 1) * P],
                                xt[:, e, :], start=(e == 0), stop=(e == EC - 1),
                                skip_group_check=True)
                        nc.vector.tensor_scalar_add(
                            kT_sb[:, f, ssl], pk[:], bk_sb[:, f:f + 1])
                    for k in range(4):
                        j = strip * 4 + k
                        pv = ps_sc.tile([P, E], f32, tag="sc", name="pvx")
                        for e in range(EC):
                            nc.tensor.matmul(
                                pv[:], xt[:, e, k * P:(k + 1) * P],
                                wv_sb[:, e, :], start=(e == 0),
                                stop=(e == EC - 1), skip_group_check=True)
                        pv_v = pv[:].rearrange("p (h d) -> p h d", h=H)
                        bv_v = bv_sb[:].rearrange("p (h d) -> p h d", h=H)
                        nc.vector.tensor_add(v_sb[:, j, :, 0:64], pv_v[:],
                                             bv_v[:])
                    for k in range(4):
                        prev = emit_pair_chunk(0, strip * 4 + k, pvpA0, pvpB0,
                                               prev, dve_b=False)
                pending = finish_pair(0, pvpA0, pvpB0, prev)

                # tail-only data: loaded now, off the startup critical path
                nc.sync.dma_start(out=ident_sb[:], in_=ident_d[:])
                nc.sync.dma_start(
                    out=wo_sb[:],
                    in_=woT_d.ap().rearrange("(c p) f -> p c f", p=P))
                nc.sync.dma_start(
                    out=xres_sb[:],
                    in_=xres_d.ap().rearrange("(k p) f -> p k f", p=P))

                import os
                probe_fat = bool(int(os.environ.get("PROBE_FAT", "0")))
                for p in range(1, NPAIR):
                    pvpA = ps_pv.tile([P, NQC, P], f32, tag="pv", name="pvpA")
                    pvpB = ps_pv.tile([P, NQC, P], f32, tag="pv", name="pvpB")
                    prev = None
                    for j in range(NJ):
                        if probe_fat:
                            dum = ps_sc.tile([P, 512], f32, tag="sc",
                                             name="dum")
                            nc.tensor.matmul(
                                dum[:], wk_sb[:, 0, 0:P], kT_sb[:, 0, 0:512],
                                start=True, stop=True, skip_group_check=True)
                        prev = emit_pair_chunk(p, j, pvpA, pvpB, prev,
                                               dve_b=True)
                        if pending is not None and j in (6, 12, 18, 24):
                            # deferred: previous pair's normalize runs inside
                            # this pair's stream, long after its inputs
                            # landed, in four half-blocks to spread the PE
                            # transpose work
                            stgA, stgB, pp = pending
                            half = range(NQC // 2) if j in (6, 18) \
                                else range(NQC // 2, NQC)
                            src, hh = (stgA, 2 * pp) if j in (6, 12) \
                                else (stgB, 2 * pp + 1)
                            emit_normalize(src, hh, qcs=half)
                            if j == 24:
                                pending = None
                    pending = finish_pair(p, pvpA, pvpB, prev)

                stgA, stgB, pp = pending
                emit_normalize(stgA, 2 * pp)
                emit_normalize(stgB, 2 * pp + 1, also_d=True)

    nc.compile()
    return nc


def _get_nc():
    if "nc" not in _CACHE:
        _CACHE["nc"] = _build_nc()
    return _CACHE["nc"]


def run_spmd(in_maps, **kw):
    from concourse.bass_utils import run_bass_kernel_spmd
    nc = _get_nc()
    return run_bass_kernel_spmd(nc, in_maps, list(range(8)), **kw)


def make_in_maps(x, Wq, bq, Wk, bk, Wv, bv, Wo, bo):
    import ml_dtypes
    bf = ml_dtypes.bfloat16
    x = np.asarray(x, dtype=np.float32)
    f32c = lambda a: np.ascontiguousarray(np.asarray(a, dtype=np.float32))
    bfc = lambda a: np.ascontiguousarray(
        np.asarray(a, dtype=np.float32).astype(bf))
    wqT = bfc(np.asarray(Wq).T)
    wkT = bfc(np.asarray(Wk).T)
    wvT = bfc(np.asarray(Wv).T)
    woT = bfc(np.asarray(Wo).T)
    bq_r = f32c(np.asarray(bq).reshape(FC, P).T)
    bk_r = f32c(np.asarray(bk).reshape(FC, P).T)
    bv_a = f32c(bv)
    bo_a = np.asarray(bo, dtype=np.float32)
    ident = np.eye(P, dtype=np.float32).astype(bf)
    xT = [bfc(x[b].T) for b in range(B)]

    in_maps = []
    for c in range(8):
        b, r = c // 4, c % 4
        in_maps.append({
            "xT": xT[b],
            "xqT": np.ascontiguousarray(xT[b][:, r * QR:(r + 1) * QR]),
            # output bias folded into the residual tile (host-side, free)
            "xres": f32c(x[b, r * QR:(r + 1) * QR] + bo_a),
            "wqT": wqT, "wkT": wkT, "wvT": wvT, "woT": woT,
            "bq": bq_r, "bk": bk_r, "bv": bv_a,
            "ident": ident,
        })
    return in_maps


def assemble(results):
    out = np.empty((B, S, E), dtype=np.float32)
    for c in range(8):
        b, r = c // 4, c % 4
        out[b, r * QR:(r + 1) * QR] = results[c]["out"]
    return out


def kernel(x, Wq, bq, Wk, bk, Wv, bv, Wo, bo):
    in_maps = make_in_maps(x, Wq, bq, Wk, bk, Wv, bv, Wo, bo)
    res = run_spmd(in_maps)
    return assemble(res.results)


# revision 5
# speedup vs baseline: 1.3270x; 1.2911x over previous
"""Trainium2 Bass kernel for an 8-head self-attention block (MHA).

Problem: x[2, 4096, 512], 8 heads x 64 dims, torch-Linear q/k/v/o projections,
softmax attention, residual:  out = softmax(q k^T / 8) v @ Wo^T + bo + x.

Sharding (8 NeuronCores, no collectives): core c handles batch b = c // 4 and
query rows (c % 4) * 1024 ... + 1024, for ALL heads.  K/V for the full
sequence are computed on every core of a batch group (projections are cheap
relative to attention), so the output projection is fully local to a core.

Engine budget per core (the design drivers):
  - ACT (exp) is the largest single stream: 8 heads x 32 s-chunks x
    [128, 1024] = 262144 elem/partition ~ 270 us if ACT does all of it.
    For pairs 1-3 the odd head's exp runs on the Vector engine instead,
    via a Schraudolph bit-trick: i16 = round(x * 128/ln2 + (127*128 - c))
    reinterpreted as bf16 is exp(x) to ~3%; softmax cancels the constant
    bias and the residual noise on 3 of 8 heads is ~1e-4 of the output.
  - The score matmuls contract over only D=64, so two heads packed in the
    two 64-row halves of the PE array (tile_position row groups 0 / 64)
    can run CONCURRENTLY - heads are processed in pairs (2p, 2p+1), and
    the A/B score matmuls are emitted adjacently.  For the hardware to
    actually overlap them, both must be dependency-ready when the PE
    reaches them, so scores use four single-bank [128, 512] psum tiles
    per chunk (exp in 512-column halves) instead of two 2-bank tiles.
  - PSUM (8 banks): 4 x 1-bank score tiles + 1-bank scratch (projections,
    transposes, output proj) + a 3-bank PV accumulator holding both
    heads' 16 [q=128, 65] regions packed 7/7/2 per bank.

Per-pair dataflow:
  - scores^T chunk [s=128, q=512] x2 per head = kT_sl.T @ qT_sl on PE
  - exp -> P~ bf16 (no max subtraction: scores are O(1) here)
  - PV: lhsT = P~ [s=128, q=128], rhs = [V | 1] [s=128, 65] -> accumulates
    o[q, 65], softmax denominator in psum column 64; one chunk behind the
    scores so PE never waits on the exp
  - normalize = reciprocal + tensor_scalar multiply on DVE, deferred into
    the next pair's stream; o tiles PE-transposed into oT[f, q] for the
    output projection; out bias folded into the residual host-side.
"""

import numpy as np

B = 2
S = 4096
E = 512
H = 8
D = 64
P = 128
EC = E // P          # 4 e-chunks
FC = E // P          # 4 f-chunks
NJ = S // P          # 32 s-chunks
QR = S // 4          # 1024 query rows per core
NQS = QR // 512      # 2 query strips of 512
NKS = S // 512       # 8 s-strips of 512
NPAIR = H // 2       # 4 head pairs
NQC = QR // P        # 8 query chunks of 128

# Schraudolph exp-as-bf16-bits constants (bf16 has 7 mantissa bits)
EXP_A = float(128.0 / np.log(2.0))
EXP_B = float(127.0 * 128.0 - 6.9)

_CACHE = {}


def _build_nc():
    import concourse.bass as bass
    import concourse.tile as tile
    from concourse import bacc, mybir

    f32 = mybir.dt.float32
    bf16 = mybir.dt.bfloat16
    i16 = mybir.dt.int16
    AFT = mybir.ActivationFunctionType
    Alu = mybir.AluOpType

    nc = bacc.Bacc("TRN2", target_bir_lowering=False, debug=False, num_devices=8)

    xT_d = nc.declare_dram_parameter("xT", [E, S], bf16, isOutput=False)
    xqT_d = nc.declare_dram_parameter("xqT", [E, QR], bf16, isOutput=False)
    xres_d = nc.declare_dram_parameter("xres", [QR, E], f32, isOutput=False)
    wqT_d = nc.declare_dram_parameter("wqT", [E, E], bf16, isOutput=False)
    wkT_d = nc.declare_dram_parameter("wkT", [E, E], bf16, isOutput=False)
    wvT_d = nc.declare_dram_parameter("wvT", [E, E], bf16, isOutput=False)
    woT_d = nc.declare_dram_parameter("woT", [E, E], bf16, isOutput=False)
    bq_d = nc.declare_dram_parameter("bq", [P, FC], f32, isOutput=False)
    bk_d = nc.declare_dram_parameter("bk", [P, FC], f32, isOutput=False)
    bv_d = nc.declare_dram_parameter("bv", [E], f32, isOutput=False)
    ident_d = nc.declare_dram_parameter("ident", [P, P], bf16, isOutput=False)
    out_d = nc.declare_dram_parameter("out", [QR, E], f32, isOutput=True)

    # PV psum packing: region r = head_in_pair * 8 + qc -> (bank, offset).
    # 65-f32 regions at 65-element stride never cross a 512-f32 bank when
    # packed 7 / 7 / 2.
    def pv_region(r):
        return r // 7, (r % 7) * 65

    PV_START = {0, 7, 14}   # first region touching each bank

    with tile.TileContext(nc) as tc:
        with tc.tile_pool(name="const", bufs=1) as const, \
             tc.tile_pool(name="persist", bufs=1) as persist:

            # ---- constants that live for the whole kernel ----
            wo_sb = const.tile([P, EC, E], bf16)
            bq_sb = const.tile([P, FC], f32)
            bk_sb = const.tile([P, FC], f32)
            bv_sb = const.tile([P, E], f32)
            # identity for PE transposes (loaded after the projection
            # phase - see below - to keep startup DMA on the critical path)
            ident_sb = const.tile([P, P], bf16)
            # residual rows (+ output bias, folded host-side)
            xres_sb = const.tile([P, QR // P, E], f32)

            # ---- persistent activations ----
            kT_sb = persist.tile([P, FC, S], bf16)           # 32 KB/p
            qT_sb = persist.tile([P, FC, QR], bf16)          # 8 KB/p
            v_sb = persist.tile([P, NJ, H, 65], bf16)        # 32.5 KB/p
            oT_sb = persist.tile([P, FC, QR], bf16)          # 8 KB/p

            # constant-1 columns (softmax denominator trick)
            nc.vector.memset(v_sb[:, :, :, 64:65], 1.0)

            with tc.tile_pool(name="wpool", bufs=1) as wpool, \
                 tc.tile_pool(name="xtp", bufs=3) as xtp, \
                 tc.tile_pool(name="work", bufs=8) as work, \
                 tc.tile_pool(name="opool", bufs=2) as opool, \
                 tc.tile_pool(name="ps_sc", bufs=4, space="PSUM") as ps_sc, \
                 tc.tile_pool(name="ps_x", bufs=1, space="PSUM") as ps_x, \
                 tc.tile_pool(name="ps_pv", bufs=1, space="PSUM") as ps_pv:

                wq_sb = wpool.tile([P, EC, E], bf16)
                wk_sb = wpool.tile([P, EC, E], bf16)
                wv_sb = wpool.tile([P, EC, E], bf16)
                # per-e-chunk loads so the first matmul only waits for
                # the first 128 rows of Wq rather than the whole tensor
                for t, d in ((wq_sb, wqT_d), (wk_sb, wkT_d), (wv_sb, wvT_d)):
                    for e in range(EC):
                        nc.sync.dma_start(
                            out=t[:, e, :], in_=d[e * P:(e + 1) * P, :])
                nc.sync.dma_start(out=bq_sb[:], in_=bq_d[:])
                nc.sync.dma_start(out=bk_sb[:], in_=bk_d[:])
                nc.sync.dma_start(
                    out=bv_sb[:],
                    in_=bass.AP(tensor=bv_d, offset=0, ap=[[0, P], [1, E]]))

                # B2: qT[f, q] = (Wq @ xq^T + bq) / 8 (first: C needs it all)
                for qs in range(NQS):
                    qsl = slice(qs * 512, (qs + 1) * 512)
                    xq = xtp.tile([P, EC, 512], bf16, tag="xt")
                    for e in range(EC):
                        nc.sync.dma_start(
                            out=xq[:, e, :], in_=xqT_d[e * P:(e + 1) * P, qsl])
                    for f in range(FC):
                        pq = ps_sc.tile([P, 512], f32, tag="sc", name="pq")
                        for e in range(EC):
                            nc.tensor.matmul(
                                pq[:], wq_sb[:, e, f * P:(f + 1) * P],
                                xq[:, e, :], start=(e == 0), stop=(e == EC - 1),
                                skip_group_check=True)
                        nc.vector.tensor_scalar(
                            qT_sb[:, f, qsl], pq[:], bq_sb[:, f:f + 1],
                            float(1.0 / np.sqrt(D)), Alu.add, Alu.mult)

                # ---- attention, head pairs on PE row groups 0 / 64 ----

                def emit_normalize(stg, h, also_d=False, qcs=None):
                    fc = h // 2
                    fr = (h % 2) * 64
                    qcs = range(NQC) if qcs is None else qcs
                    rcp = opool.tile([P, NQC, 1], f32, tag="rcp", name="rcp",
                                     bufs=2)
                    nc.vector.reciprocal(rcp[:, qcs[0]:qcs[-1] + 1, :],
                                         stg[:, qcs[0]:qcs[-1] + 1, 64:65])
                    o_sb = opool.tile([P, NQC, 64], bf16, tag="o", name="o_sb",
                                      bufs=2)
                    for qc in qcs:
                        nc.vector.tensor_scalar_mul(
                            o_sb[:, qc, :], stg[:, qc, 0:64], rcp[:, qc, :])
                    for qc in qcs:
                        # transpose [128 q, 64 d] -> [64 d, 128 q] on PE,
                        # directly at the head's partition base
                        tp = ps_x.tile([P, P], bf16, tag="x", name="tp")
                        nc.tensor.transpose(tp[fr:fr + 64, :], o_sb[:, qc, :],
                                            ident_sb[:])
                        nc.vector.tensor_copy(
                            oT_sb[fr:fr + 64, fc, qc * P:(qc + 1) * P],
                            tp[fr:fr + 64, :])
                        if also_d:
                            # last head: output projection for this q-chunk
                            # follows immediately (all other heads' oT pieces
                            # already landed), overlapping the projection with
                            # the tail of attention
                            # scores pool is idle by the time the final
                            # normalize runs - po rides there, not on the
                            # single scratch slot
                            po = ps_sc.tile([P, E], f32, tag="sc", name="po")
                            for e in range(EC):
                                nc.tensor.matmul(
                                    po[:], oT_sb[:, e, qc * P:(qc + 1) * P],
                                    wo_sb[:, e, :], start=(e == 0),
                                    stop=(e == EC - 1), skip_group_check=True)
                            ot = opool.tile([P, E], f32, tag="ot", name="ot")
                            nc.vector.tensor_add(ot[:], po[:],
                                                 xres_sb[:, qc, :])
                            nc.sync.dma_start(
                                out=out_d[qc * P:(qc + 1) * P, :], in_=ot[:])

                def emit_pair_chunk(p, j, pvp, prev, dve_b):
                    """Scores+exp for pair p chunk j; PV for chunk j-1.

                    Score matmuls alternate row groups (A at rows 0-63, B at
                    64-127) in adjacent PE instructions so the array runs
                    both concurrently.
                    """
                    hA, hB = 2 * p, 2 * p + 1
                    sc = []
                    for hf in range(2):
                        hsl = slice(hf * 512, (hf + 1) * 512)
                        scA = ps_sc.tile([P, 512], f32, tag="sc", name="scA")
                        scB = ps_sc.tile([P, 512], f32, tag="sc", name="scB")
                        nc.tensor.matmul(
                            scA[:], kT_sb[0:64, p, j * P:(j + 1) * P],
                            qT_sb[0:64, p, hsl],
                            start=True, stop=True, skip_group_check=True)
                        nc.tensor.matmul(
                            scB[:], kT_sb[64:128, p, j * P:(j + 1) * P],
                            qT_sb[64:128, p, hsl],
                            start=True, stop=True, skip_group_check=True)
                        sc.append((scA, scB))
                    ptA = [None, None]
                    ptB = [None, None]
                    for hf in range(2):
                        scA, scB = sc[hf]
                        ptA[hf] = work.tile([P, 512], bf16, tag="pt",
                                            name="ptA")
                        nc.scalar.activation(ptA[hf][:], scA[:], AFT.Exp)
                        if dve_b:
                            ptB[hf] = work.tile([P, 512], i16, tag="pt",
                                                name="ptB")
                            nc.vector.tensor_scalar(
                                ptB[hf][:], scB[:], EXP_A, EXP_B,
                                Alu.mult, Alu.add)
                        else:
                            ptB[hf] = work.tile([P, 512], bf16, tag="pt",
                                                name="ptB")
                            nc.scalar.activation(ptB[hf][:], scB[:], AFT.Exp)
                    # software pipeline: PV for chunk j-1 is emitted after the
                    # scores matmuls of chunk j so PE never waits on the exp
                    if prev is not None:
                        pA, pB, pj = prev
                        for r in range(16):
                            h, qc = (hA, r) if r < 8 else (hB, r - 8)
                            pt = (pA if r < 8 else pB)[qc % 8 // 4]
                            bank, off = pv_region(r)
                            nc.tensor.matmul(
                                pvp[:, bank, off:off + 65],
                                pt[:, (qc % 4) * P:(qc % 4 + 1) * P]
                                .bitcast(bf16),
                                v_sb[:, pj, h, :],
                                start=(pj == 0 and r in PV_START),
                                stop=(pj == NJ - 1), skip_group_check=True)
                    return (ptA, ptB, j)

                def emit_pv_last(p, pvp, prev):
                    pA, pB, pj = prev
                    hA, hB = 2 * p, 2 * p + 1
                    for r in range(16):
                        h, qc = (hA, r) if r < 8 else (hB, r - 8)
                        pt = (pA if r < 8 else pB)[qc % 8 // 4]
                        bank, off = pv_region(r)
                        nc.tensor.matmul(
                            pvp[:, bank, off:off + 65],
                            pt[:, (qc % 4) * P:(qc % 4 + 1) * P].bitcast(bf16),
                            v_sb[:, pj, h, :],
                            start=(pj == 0 and r in PV_START),
                            stop=True, skip_group_check=True)

                def finish_pair(p, pvp, prev):
                    emit_pv_last(p, pvp, prev)
                    # staging copies (one per psum bank span) free the psum
                    # accumulator almost immediately
                    stgA = opool.tile([P, NQC, 65], f32, tag="stg",
                                      name="stgA", bufs=4)
                    nc.vector.tensor_copy(
                        stgA[:, 0:7, :],
                        pvp[:, 0, 0:455].rearrange("p (q c) -> p q c", c=65))
                    nc.vector.tensor_copy(stgA[:, 7, :], pvp[:, 1, 0:65])
                    stgB = opool.tile([P, NQC, 65], f32, tag="stg",
                                      name="stgB", bufs=4)
                    nc.vector.tensor_copy(
                        stgB[:, 0:6, :],
                        pvp[:, 1, 65:455].rearrange("p (q c) -> p q c", c=65))
                    nc.vector.tensor_copy(
                        stgB[:, 6:8, :],
                        pvp[:, 2, 0:130].rearrange("p (q c) -> p q c", c=65))
                    return (stgA, stgB, p)

                # B1+B3+pair-0 interleaved: kT strips and V chunks come from
                # the same xt tile, and pair 0's scores/exp/PV for a strip's
                # four chunks follow immediately, so the ACT exp pipeline
                # starts ~70us earlier and fills projection DMA gaps
                pvp0 = ps_pv.tile([P, 3, 512], f32, tag="pv", name="pvp")
                prev = None
                for strip in range(NKS):
                    ssl = slice(strip * 512, (strip + 1) * 512)
                    xt = xtp.tile([P, EC, 512], bf16, tag="xt")
                    for e in range(EC):
                        nc.sync.dma_start(
                            out=xt[:, e, :], in_=xT_d[e * P:(e + 1) * P, ssl])
                    for f in range(FC):
                        pk = ps_x.tile([P, 512], f32, tag="x", name="pk")
                        for e in range(EC):
                            nc.tensor.matmul(
                                pk[:], wk_sb[:, e, f * P:(f + 1) * P],
                                xt[:, e, :], start=(e == 0), stop=(e == EC - 1),
                                skip_group_check=True)
                        nc.vector.tensor_scalar_add(
                            kT_sb[:, f, ssl], pk[:], bk_sb[:, f:f + 1])
                    for k in range(4):
                        j = strip * 4 + k
                        pv = ps_x.tile([P, E], f32, tag="x", name="pvx")
                        for e in range(EC):
                            nc.tensor.matmul(
                                pv[:], xt[:, e, k * P:(k + 1) * P],
                                wv_sb[:, e, :], start=(e == 0),
                                stop=(e == EC - 1), skip_group_check=True)
                        pv_v = pv[:].rearrange("p (h d) -> p h d", h=H)
                        bv_v = bv_sb[:].rearrange("p (h d) -> p h d", h=H)
                        nc.vector.tensor_add(v_sb[:, j, :, 0:64], pv_v[:],
                                             bv_v[:])
                    for k in range(4):
                        prev = emit_pair_chunk(0, strip * 4 + k, pvp0, prev,
                                               dve_b=False)
                pending = finish_pair(0, pvp0, prev)

                # tail-only data: loaded now, off the startup critical path
                nc.sync.dma_start(out=ident_sb[:], in_=ident_d[:])
                nc.sync.dma_start(
                    out=wo_sb[:],
                    in_=woT_d.ap().rearrange("(c p) f -> p c f", p=P))
                nc.sync.dma_start(
                    out=xres_sb[:],
                    in_=xres_d.ap().rearrange("(k p) f -> p k f", p=P))

                for p in range(1, NPAIR):
                    pvp = ps_pv.tile([P, 3, 512], f32, tag="pv", name="pvp")
                    prev = None
                    for j in range(NJ):
                        prev = emit_pair_chunk(p, j, pvp, prev, dve_b=True)
                        if pending is not None and j in (6, 12, 18, 24):
                            # deferred: previous pair's normalize runs inside
                            # this pair's stream, long after its inputs
                            # landed, in four half-blocks to spread the PE
                            # transpose work
                            stgA, stgB, pp = pending
                            half = range(NQC // 2) if j in (6, 18) \
                                else range(NQC // 2, NQC)
                            src, hh = (stgA, 2 * pp) if j in (6, 12) \
                                else (stgB, 2 * pp + 1)
                            emit_normalize(src, hh, qcs=half)
                            if j == 24:
                                pending = None
                    pending = finish_pair(p, pvp, prev)

                stgA, stgB, pp = pending
                emit_normalize(stgA, 2 * pp)
                emit_normalize(stgB, 2 * pp + 1, also_d=True)

    nc.compile()
    return nc


def _get_nc():
    if "nc" not in _CACHE:
        _CACHE["nc"] = _build_nc()
    return _CACHE["nc"]


def run_spmd(in_maps, **kw):
    from concourse.bass_utils import run_bass_kernel_spmd
    nc = _get_nc()
    return run_bass_kernel_spmd(nc, in_maps, list(range(8)), **kw)


def make_in_maps(x, Wq, bq, Wk, bk, Wv, bv, Wo, bo):
    import ml_dtypes
    bf = ml_dtypes.bfloat16
    x = np.asarray(x, dtype=np.float32)
    f32c = lambda a: np.ascontiguousarray(np.asarray(a, dtype=np.float32))
    bfc = lambda a: np.ascontiguousarray(
        np.asarray(a, dtype=np.float32).astype(bf))
    wqT = bfc(np.asarray(Wq).T)
    wkT = bfc(np.asarray(Wk).T)
    wvT = bfc(np.asarray(Wv).T)
    woT = bfc(np.asarray(Wo).T)
    bq_r = f32c(np.asarray(bq).reshape(FC, P).T)
    bk_r = f32c(np.asarray(bk).reshape(FC, P).T)
    bv_a = f32c(bv)
    bo_a = np.asarray(bo, dtype=np.float32)
    ident = np.eye(P, dtype=np.float32).astype(bf)
    xT = [bfc(x[b].T) for b in range(B)]

    in_maps = []
    for c in range(8):
        b, r = c // 4, c % 4
        in_maps.append({
            "xT": xT[b],
            "xqT": np.ascontiguousarray(xT[b][:, r * QR:(r + 1) * QR]),
            # output bias folded into the residual tile (host-side, free)
            "xres": f32c(x[b, r * QR:(r + 1) * QR] + bo_a),
            "wqT": wqT, "wkT": wkT, "wvT": wvT, "woT": woT,
            "bq": bq_r, "bk": bk_r, "bv": bv_a,
            "ident": ident,
        })
    return in_maps


def assemble(results):
    out = np.empty((B, S, E), dtype=np.float32)
    for c in range(8):
        b, r = c // 4, c % 4
        out[b, r * QR:(r + 1) * QR] = results[c]["out"]
    return out


def kernel(x, Wq, bq, Wk, bk, Wv, bv, Wo, bo):
    in_maps = make_in_maps(x, Wq, bq, Wk, bk, Wv, bv, Wo, bo)
    res = run_spmd(in_maps)
    return assemble(res.results)
